# Initial kernel scaffold
#
"""Causal self-attention (B=2,T=2048,C=1024,H=16,hd=64) with QK-RMSNorm + RoPE.

8-core Trainium2 Bass kernel. Sharding: tensor-parallel over heads (2 heads per
core) for QKV + attention, then an AllToAll reshards the attention output
token-wise so each core computes the exact c_proj output for its 512-token
slice (no partial sums, no all-reduce).

Layout strategy: everything feature-major ("transposed") on device.
  - host feeds xT [C, B*T]; per-core waT = w_attn[sel_rows].T so QKV matmuls
    produce qT/kT/vT [feat, tok] with no on-device activation transposes.
  - q,k feature order is permuted to [evens, odds] per head (host-side weight
    row permutation) which turns interleaved RoPE into half-block ops; S = q.k
    is invariant to the shared permutation.
  - S^T tiles [keys,queries] come from lhsT=kT, rhs=qT; softmax denominator is
    computed by a ones-column appended to V (scores are bounded: |s| <= 8
    after RMS-norm, so exp needs no max subtraction).
"""

import numpy as np

import concourse.bass as bass
import concourse.mybir as mybir
import concourse.tile as tile
from concourse import bacc
from concourse.bass_utils import run_bass_kernel_spmd

B, T, C = 2, 2048, 1024
H, HD = 16, 64
N_CORES = 8
HPC = H // N_CORES  # heads per core = 2
BT = B * T  # 4096 flattened tokens
FPC = HPC * HD  # feats per core = 128
EPS = 1e-6
TN = BT // 512  # 8 token tiles of 512
QB = T // 512  # 4 query blocks per sequence

f32 = mybir.dt.float32
f32r = mybir.dt.float32r
bf16 = mybir.dt.bfloat16
MUL = mybir.AluOpType.mult
ADD = mybir.AluOpType.add
AF = mybir.ActivationFunctionType

RG = [list(range(N_CORES))]


def r32(ap):
    return ap.bitcast(f32r)


def build_nc(single_core=False, no_cc=False):
    no_cc = no_cc or single_core
    nc = bacc.Bacc("TRN2", target_bir_lowering=False, debug=False,
                   num_devices=1 if single_core else N_CORES)

    xT = nc.dram_tensor("xT", [C, BT], bf16, kind="ExternalInput")
    waT = nc.dram_tensor("waT", [C, 3 * FPC], bf16, kind="ExternalInput")
    wpT = nc.dram_tensor("wpT", [C, C], bf16, kind="ExternalInput")
    csT = nc.dram_tensor("csT", [128, T], f32, kind="ExternalInput")
    snT = nc.dram_tensor("snT", [128, T], f32, kind="ExternalInput")
    qw = nc.dram_tensor("qw", [128, 1], f32, kind="ExternalInput")
    kw = nc.dram_tensor("kw", [128, 1], f32, kind="ExternalInput")
    bones = nc.dram_tensor("bones", [128, 2], f32, kind="ExternalInput")
    sel2 = nc.dram_tensor("sel2", [2, 128], f32, kind="ExternalInput")
    wedge = nc.dram_tensor("wedge", [128, 128], f32, kind="ExternalInput")
    ident = nc.dram_tensor("ident", [128, 128], f32, kind="ExternalInput")
    vones = nc.dram_tensor("vones", [128, 32], f32, kind="ExternalInput")
    out = nc.dram_tensor("out", [BT // N_CORES, C], f32, kind="ExternalOutput")

    with tile.TileContext(nc) as tc:
        with (
            tc.tile_pool(name="const", bufs=1) as const,
            tc.tile_pool(name="resid", bufs=1) as resid,
            tc.tile_pool(name="xtp", bufs=6) as xtp,
            tc.tile_pool(name="work", bufs=3) as work,
            tc.tile_pool(name="pwork", bufs=4) as pwork,
            tc.tile_pool(name="mm", bufs=2, space="PSUM") as mmp,
            tc.tile_pool(name="yp", bufs=2, space="PSUM") as ypp,
            tc.tile_pool(name="sp", bufs=1, space="PSUM") as spp,
            tc.tile_pool(name="bcp", bufs=1, space="PSUM") as bcp,
            tc.tile_pool(name="dram", bufs=1, space="DRAM") as dramp,
        ):
            # ---- constants to SBUF ----
            wa_sb = const.tile([128, C // 128, 3 * FPC], bf16, tag="wa")
            nc.sync.dma_start(wa_sb[:], waT.ap().rearrange("(o p) f -> p o f", p=128))
            qw_sb = const.tile([128, 1], f32, tag="qw")
            nc.sync.dma_start(qw_sb[:], qw[:, :])
            kw_sb = const.tile([128, 1], f32, tag="kw")
            nc.sync.dma_start(kw_sb[:], kw[:, :])
            bo_sb = const.tile([128, 2], f32r, tag="bo")
            nc.sync.dma_start(bo_sb[:], r32(bones[:, :]))
            s2_sb = const.tile([2, 128], f32r, tag="s2")
            nc.sync.dma_start(s2_sb[:], r32(sel2[:, :]))
            id_sb = const.tile([128, 128], f32, tag="id")
            nc.sync.dma_start(id_sb[:], ident[:, :])
            eps_sb = const.tile([128, 1], f32, tag="eps")
            nc.vector.memset(eps_sb[:], EPS)
            cs_sb = const.tile([128, T], f32, tag="cs")
            sn_sb = const.tile([128, T], f32, tag="sn")
            wg_sb = const.tile([128, 128], f32, tag="wg")

            def emit_late_consts():
                nc.sync.dma_start(vA[:, :, HD], r32(vones[:, :]))
                nc.sync.dma_start(vA[:, :, 2 * HD + 1], r32(vones[:, :]))
                nc.sync.dma_start(cs_sb[:], csT[:, :])
                nc.sync.dma_start(sn_sb[:], snT[:, :])
                nc.sync.dma_start(wg_sb[:], wedge[:, :])

            # ---- residents ----
            qT = resid.tile([128, BT], f32r, tag="qT")   # roped+normed q^T
            kT = resid.tile([128, BT], f32r, tag="kT")
            # attention out^T, both heads packed [128, BT]; written via
            # SBUF->SBUF DMA (cross-partition moves are DMA-only)
            yHp = resid.tile([128, BT], bf16, tag="yHp")
            # V in token-major + ones cols: per head h: cols [65h:65h+64]=V_h,
            # col 65h+64 = 1.0
            vA = resid.tile([128, BT // 128, 2 * (HD + 1)], f32r, tag="vA")

            xT_r = xT.ap().rearrange("(o p) t -> p o t", p=128)

            # ================= QKV + RMSNorm + RoPE =================
            xts = {}

            def emit_xt(n):
                tok = slice(512 * n, 512 * n + 512)
                xtA = xtp.tile([128, 4, 512], bf16, tag="xt", name=f"xtA{n}")
                nc.sync.dma_start(xtA[:], xT_r[:, 0:4, tok])
                xtB = xtp.tile([128, 4, 512], bf16, tag="xt", name=f"xtB{n}")
                nc.sync.dma_start(xtB[:], xT_r[:, 4:8, tok])
                xts[n] = (xtA, xtB)

            def emit_qkv(n):
                tok = slice(512 * n, 512 * n + 512)
                ct = slice(512 * (n % 4), 512 * (n % 4) + 512)
                if n not in xts:
                    emit_xt(n)
                xtA, xtB = xts.pop(n)

                bigQK = mmp.tile([128, 1024], f32, tag="big", name=f"qk{n}")
                bigV = mmp.tile([128, 1024], f32, tag="big", name=f"v{n}")
                for m, dst, wcol in ((0, qT, qw_sb), (1, kT, kw_sb), (2, None, None)):
                    ps = bigV[:, 0:512] if m == 2 else bigQK[:, 512 * m:512 * m + 512]
                    for kt in range(C // 128):
                        nc.tensor.matmul(
                            ps,
                            wa_sb[:, kt, 128 * m:128 * m + 128],
                            xtA[:, kt, :] if kt < 4 else xtB[:, kt - 4, :],
                            start=(kt == 0), stop=(kt == C // 128 - 1),
                        )
                    if m == 2:
                        # V: token-major via PE transpose of 128x128 blocks
                        vs = work.tile([128, 512], f32, tag="vs", name=f"vs{n}")
                        nc.scalar.copy(vs[:], ps)
                        for j in range(4):
                            pt = spp.tile([128, 128], f32, tag="sm", name=f"vt{n}_{j}")
                            nc.tensor.transpose(pt[:], vs[:, 128 * j:128 * j + 128],
                                                id_sb[:])
                            kt_g = 4 * n + j
                            nc.vector.tensor_copy(
                                vA[:, kt_g].rearrange("p (h d) -> p h d", h=2)[:, :, 0:HD],
                                pt[:, :].rearrange("p (h d) -> p h d", h=2))
                        continue

                    # stats from raw (pre-weight) psum
                    sq = work.tile([128, 512], f32, tag="scr", name=f"sq{n}_{m}")
                    nc.scalar.activation(r32(sq[:]), ps, AF.Square)
                    ss = spp.tile([2, 512], f32, tag="sm", name=f"ss{n}_{m}")
                    nc.tensor.matmul(ss[:], r32(bo_sb[:]), r32(sq[:]),
                                     start=True, stop=True)
                    inv = work.tile([2, 512], f32, tag="rms", name=f"rms{n}_{m}")
                    nc.scalar.activation(r32(inv[:]), ss[:], AF.Sqrt,
                                         bias=eps_sb[0:2, :], scale=1.0 / HD)
                    with nc.allow_low_precision(reason="f32r is fp32-width"):
                        nc.vector.reciprocal(r32(inv[:]), inv[:])

                    # apply norm weight on the way out of PSUM
                    nc.vector.tensor_scalar_mul(dst[:, tok], ps, wcol[:])

                    # rope: r = q*CS + swap(q)*SN  (swap halves within head)
                    sw = work.tile([128, 512], f32r, tag="sw", name=f"sw{n}_{m}")
                    for h in range(HPC):
                        b0 = 64 * h
                        nc.sync.dma_start(sw[b0:b0 + 32, :], dst[b0 + 32:b0 + 64, tok])
                        nc.sync.dma_start(sw[b0 + 32:b0 + 64, :], dst[b0:b0 + 32, tok])
                    nc.gpsimd.tensor_tensor(sw[:], sw[:], sn_sb[:, ct], MUL)
                    nc.vector.tensor_tensor(dst[:, tok], dst[:, tok], cs_sb[:, ct], MUL)
                    nc.vector.tensor_tensor(dst[:, tok], dst[:, tok], sw[:], ADD)

                    # apply 1/rms: broadcast [2,512] -> [128,512] via K=2 matmul
                    bc = bcp.tile([128, 512], f32, tag="bc", name=f"bc{n}_{m}")
                    nc.tensor.matmul(bc[:], r32(s2_sb[:]), r32(inv[:]),
                                     start=True, stop=True)
                    nc.vector.tensor_tensor(r32(dst[:, tok]), dst[:, tok], bc[:], MUL)

            # ================= causal attention =================
            a_in = dramp.tile([N_CORES, 128, 512], bf16, tag="a_in")

            def emit_attn(b, i):
                if True:
                    qcol = slice(2048 * b + 512 * i, 2048 * b + 512 * i + 512)
                    nkt = 4 * i + 4
                    yps = [ypp.tile([HD + 1, 512], f32, tag="y",
                                    name=f"y{b}_{i}_{h}") for h in range(HPC)]
                    for kt in range(nkt):
                        qs = 128 * (kt - 4 * i) if kt >= 4 * i else 0
                        kc = 2048 * b + 128 * kt
                        kt_g = 16 * b + kt
                        sps = mmp.tile([128, 1024], f32, tag="big",
                                       name=f"s{b}_{i}_{kt}")
                        pT = pwork.tile([128, 1024], f32, tag="pT",
                                        name=f"p{b}_{i}_{kt}")
                        for h in range(HPC):
                            hb = 64 * h
                            nc.tensor.matmul(
                                sps[:, 512 * h + qs:512 * h + 512],
                                r32(kT[hb:hb + 64, kc:kc + 128]),
                                r32(qT[hb:hb + 64, qcol][:, qs:]),
                                start=True, stop=True,
                                tile_position=(hb, 0),
                            )
                        sps3 = sps[:, :].rearrange("p (h q) -> p h q", h=2)[:, :, qs:]
                        pT3 = pT[:, :].rearrange("p (h q) -> p h q", h=2)[:, :, qs:]
                        nc.scalar.activation(r32(pT3), sps3, AF.Exp,
                                             scale=1.0 / 8.0)
                        for h in range(HPC):
                            if kt >= 4 * i:
                                nc.gpsimd.tensor_tensor(
                                    r32(pT[:, 512 * h + qs:512 * h + qs + 128]),
                                    pT[:, 512 * h + qs:512 * h + qs + 128],
                                    wg_sb[:], MUL)
                            nc.tensor.matmul(
                                yps[h][:, qs:],
                                r32(vA[:, kt_g, (HD + 1) * h:(HD + 1) * h + HD + 1]),
                                r32(pT[:, 512 * h + qs:512 * h + 512]),
                                start=(kt == 0), stop=(kt == nkt - 1),
                            )
                    # normalize by the ones-column denominator
                    for h in range(HPC):
                        di = work.tile([1, 512], f32, tag="rms",
                                       name=f"di{b}_{i}_{h}")
                        with nc.allow_low_precision(reason="f32r is fp32-width"):
                            nc.vector.reciprocal(r32(di[:]), yps[h][HD:HD + 1, :])
                        dp = spp.tile([64, 512], f32, tag="sm",
                                      name=f"dp{b}_{i}_{h}")
                        nc.tensor.matmul(dp[:], r32(s2_sb[0:1, 0:64]), r32(di[:]),
                                         start=True, stop=True)
                        dpS = work.tile([64, 512], f32, tag="dpS",
                                        name=f"dpS{b}_{i}_{h}")
                        nc.scalar.copy(dpS[:], dp[:])
                        ybf = work.tile([HD, 512], bf16, tag="ybf",
                                        name=f"ybf{b}_{i}_{h}")
                        nc.vector.tensor_tensor(ybf[:, :],
                                                yps[h][:HD, :], dpS[:, :],
                                                MUL)
                        nc.sync.dma_start(yHp[64 * h:64 * h + HD, qcol],
                                          ybf[:, :])
                    nc.sync.dma_start(a_in[4 * b + i], yHp[:, qcol])

            emit_xt(0)
            emit_late_consts()
            emit_qkv(0)
            for n in range(1, TN // 2):
                emit_qkv(n)
                emit_attn(0, n - 1)
            wp_sb = resid.tile([128, N_CORES, 1024], bf16, tag="wp_sb")
            nc.sync.dma_start(
                wp_sb[:], wpT.ap().rearrange("(o p) f -> p o f", p=128))
            emit_qkv(TN // 2)
            emit_attn(0, 3)
            for n in range(TN // 2 + 1, TN):
                emit_qkv(n)
                emit_attn(1, n - TN // 2 - 1)
            emit_attn(1, 3)

            # ================= AllToAll reshard =================
            a_out = dramp.tile([N_CORES, 128, 512], bf16, tag="a_out")
            if no_cc:
                nc.sync.dma_start(a_out[:, :, :], a_in[:, :, :])
            else:
                nc.gpsimd.collective_compute(
                    "AllToAll", mybir.AluOpType.bypass, replica_groups=RG,
                    ins=[a_in[:, :, :].opt()], outs=[a_out[:, :, :].opt()])


            # ================= c_proj on own 512-token slice =================
            ybr = resid.tile([128, N_CORES, 512], bf16, tag="ybr")
            for r in range(N_CORES):
                nc.sync.dma_start(ybr[:, r, :], a_out[r])
            for cc in range(2):
                ccol = slice(512 * cc, 512 * cc + 512)
                bigP = [mmp.tile([128, 1024], f32, tag="big",
                                 name=f"po{cc}_{t}") for t in range(2)]
                pouts = [bigP[t // 2][:, 512 * (t % 2):512 * (t % 2) + 512]
                         for t in range(4)]
                for r in range(N_CORES):
                    for t in range(4):
                        nc.tensor.matmul(
                            pouts[t],
                            ybr[:, r, 128 * t:128 * t + 128],
                            wp_sb[:, r, ccol],
                            start=(r == 0), stop=(r == N_CORES - 1),
                        )
                for t in range(4):
                    ob = work.tile([128, 512], f32, tag="scr", name=f"ob{cc}_{t}")
                    nc.scalar.copy(ob[:], pouts[t])
                    nc.sync.dma_start(out[128 * t:128 * t + 128, ccol], ob[:])

    nc.compile()
    return nc


def make_in_maps(x, freqs_cos, freqs_sin, w_attn, w_proj, q_norm_w, k_norm_w):
    x = np.asarray(x, np.float32)
    freqs_cos = np.asarray(freqs_cos, np.float32)
    freqs_sin = np.asarray(freqs_sin, np.float32)
    w_attn = np.asarray(w_attn, np.float32)
    w_proj = np.asarray(w_proj, np.float32)
    q_norm_w = np.asarray(q_norm_w, np.float32)
    k_norm_w = np.asarray(k_norm_w, np.float32)

    perm = np.concatenate([np.arange(0, HD, 2), np.arange(1, HD, 2)])
    import ml_dtypes
    xTf = np.ascontiguousarray(x.reshape(BT, C).T.astype(ml_dtypes.bfloat16))
    import ml_dtypes as _md
    wpT = np.ascontiguousarray(w_proj.T.astype(_md.bfloat16))

    cs = np.ascontiguousarray(freqs_cos.T)  # [32, T]
    sn = np.ascontiguousarray(freqs_sin.T)
    csT = np.concatenate([cs, cs] * HPC, axis=0)          # [128, T]
    snT = np.concatenate([-sn, sn] * HPC, axis=0)

    qwc = np.tile(q_norm_w[perm], HPC)[:, None].astype(np.float32)
    kwc = np.tile(k_norm_w[perm], HPC)[:, None].astype(np.float32)

    bones = np.zeros((128, 2), np.float32)
    bones[:64, 0] = 1.0
    bones[64:, 1] = 1.0
    sel2 = np.zeros((2, 128), np.float32)
    sel2[0, :64] = 1.0
    sel2[1, 64:] = 1.0
    wedge = (np.arange(128)[:, None] <= np.arange(128)[None, :]).astype(np.float32)
    vones = np.ones((128, 32), np.float32)
    ident = np.eye(128, dtype=np.float32)

    in_maps = []
    for c in range(N_CORES):
        rows = []
        for sec in range(3):  # q, k, v sections of w_attn
            for h in (HPC * c, HPC * c + 1):
                base = C * sec + HD * h
                if sec < 2:
                    rows.append(base + perm)
                else:
                    rows.append(base + np.arange(HD))
        sel_rows = np.concatenate(rows)
        waT = np.ascontiguousarray(w_attn[sel_rows].T.astype(ml_dtypes.bfloat16))
        in_maps.append({
            "xT": xTf, "waT": waT, "wpT": wpT, "csT": csT, "snT": snT,
            "qw": qwc, "kw": kwc, "bones": bones, "sel2": sel2,
            "wedge": wedge, "ident": ident, "vones": vones,
        })
    return in_maps


_NC_CACHE = {}


def get_nc():
    if "nc" not in _NC_CACHE:
        _NC_CACHE["nc"] = build_nc()
    return _NC_CACHE["nc"]


def kernel(x, freqs_cos, freqs_sin, w_attn, w_proj, q_norm_w, k_norm_w):
    nc = get_nc()
    in_maps = make_in_maps(x, freqs_cos, freqs_sin, w_attn, w_proj,
                           q_norm_w, k_norm_w)
    res = run_bass_kernel_spmd(nc, in_maps, core_ids=list(range(N_CORES)))
    out = np.concatenate([res.results[c]["out"] for c in range(N_CORES)], axis=0)
    return out.reshape(B, T, C).astype(np.float32)



# revision 1
# speedup vs baseline: 10.8333x; 10.8333x over previous
"""Causal self-attention (B=2,T=2048,C=1024,H=16,hd=64) with QK-RMSNorm + RoPE.

8-core Trainium2 Bass kernel. Sharding: tensor-parallel over heads (2 heads per
core) for QKV + attention, then an AllToAll reshards the attention output
token-wise so each core computes the exact c_proj output for its 512-token
slice (no partial sums, no all-reduce).

Layout strategy: everything feature-major ("transposed") on device.
  - host feeds xT [C, B*T]; per-core waT = w_attn[sel_rows].T so QKV matmuls
    produce qT/kT/vT [feat, tok] with no on-device activation transposes.
  - q,k feature order is permuted to [evens, odds] per head (host-side weight
    row permutation) which turns interleaved RoPE into half-block ops; S = q.k
    is invariant to the shared permutation.
  - S^T tiles [keys,queries] come from lhsT=kT, rhs=qT; softmax denominator is
    computed by a ones-column appended to V (scores are bounded: |s| <= 8
    after RMS-norm, so exp needs no max subtraction).
"""

import numpy as np

import concourse.bass as bass
import concourse.mybir as mybir
import concourse.tile as tile
from concourse import bacc
from concourse.bass_utils import run_bass_kernel_spmd

B, T, C = 2, 2048, 1024
H, HD = 16, 64
N_CORES = 8
HPC = H // N_CORES  # heads per core = 2
BT = B * T  # 4096 flattened tokens
FPC = HPC * HD  # feats per core = 128
EPS = 1e-6
TN = BT // 512  # 8 token tiles of 512
QB = T // 512  # 4 query blocks per sequence

f32 = mybir.dt.float32
f32r = mybir.dt.float32r
bf16 = mybir.dt.bfloat16
MUL = mybir.AluOpType.mult
ADD = mybir.AluOpType.add
AF = mybir.ActivationFunctionType

RG = [list(range(N_CORES))]


def r32(ap):
    return ap.bitcast(f32r)


def build_nc(single_core=False, no_cc=False):
    no_cc = no_cc or single_core
    nc = bacc.Bacc("TRN2", target_bir_lowering=False, debug=False,
                   num_devices=1 if single_core else N_CORES)

    xT = nc.dram_tensor("xT", [C, BT], bf16, kind="ExternalInput")
    waT = nc.dram_tensor("waT", [C, 3 * FPC], bf16, kind="ExternalInput")
    wpT = nc.dram_tensor("wpT", [C, C], bf16, kind="ExternalInput")
    csT = nc.dram_tensor("csT", [128, T], f32, kind="ExternalInput")
    snT = nc.dram_tensor("snT", [128, T], f32, kind="ExternalInput")
    qw = nc.dram_tensor("qw", [128, 1], f32, kind="ExternalInput")
    kw = nc.dram_tensor("kw", [128, 1], f32, kind="ExternalInput")
    bones = nc.dram_tensor("bones", [128, 2], f32, kind="ExternalInput")
    sel2 = nc.dram_tensor("sel2", [2, 128], f32, kind="ExternalInput")
    wedge = nc.dram_tensor("wedge", [128, 128], f32, kind="ExternalInput")
    ident = nc.dram_tensor("ident", [128, 128], f32, kind="ExternalInput")
    vones = nc.dram_tensor("vones", [128, 32], f32, kind="ExternalInput")
    out = nc.dram_tensor("out", [BT // N_CORES, C], f32, kind="ExternalOutput")

    with tile.TileContext(nc) as tc:
        with (
            tc.tile_pool(name="const", bufs=1) as const,
            tc.tile_pool(name="resid", bufs=1) as resid,
            tc.tile_pool(name="xtp", bufs=6) as xtp,
            tc.tile_pool(name="work", bufs=3) as work,
            tc.tile_pool(name="pwork", bufs=4) as pwork,
            tc.tile_pool(name="mm", bufs=2, space="PSUM") as mmp,
            tc.tile_pool(name="yp", bufs=2, space="PSUM") as ypp,
            tc.tile_pool(name="sp", bufs=1, space="PSUM") as spp,
            tc.tile_pool(name="bcp", bufs=1, space="PSUM") as bcp,
            tc.tile_pool(name="dram", bufs=1, space="DRAM") as dramp,
        ):
            # ---- constants to SBUF ----
            wa_sb = const.tile([128, C // 128, 3 * FPC], bf16, tag="wa")
            nc.sync.dma_start(wa_sb[:], waT.ap().rearrange("(o p) f -> p o f", p=128))
            qw_sb = const.tile([128, 1], f32, tag="qw")
            nc.sync.dma_start(qw_sb[:], qw[:, :])
            kw_sb = const.tile([128, 1], f32, tag="kw")
            nc.sync.dma_start(kw_sb[:], kw[:, :])
            bo_sb = const.tile([128, 2], f32r, tag="bo")
            nc.sync.dma_start(bo_sb[:], r32(bones[:, :]))
            s2_sb = const.tile([2, 128], f32r, tag="s2")
            nc.sync.dma_start(s2_sb[:], r32(sel2[:, :]))
            id_sb = const.tile([128, 128], f32, tag="id")
            nc.sync.dma_start(id_sb[:], ident[:, :])
            eps_sb = const.tile([128, 1], f32, tag="eps")
            nc.vector.memset(eps_sb[:], EPS)
            cs_sb = const.tile([128, T], f32, tag="cs")
            sn_sb = const.tile([128, T], f32, tag="sn")
            wg_sb = const.tile([128, 128], f32, tag="wg")

            def emit_late_consts():
                nc.sync.dma_start(vA[:, :, HD], r32(vones[:, :]))
                nc.sync.dma_start(vA[:, :, 2 * HD + 1], r32(vones[:, :]))
                nc.sync.dma_start(cs_sb[:], csT[:, :])
                nc.sync.dma_start(sn_sb[:], snT[:, :])
                nc.sync.dma_start(wg_sb[:], wedge[:, :])

            # ---- residents ----
            qT = resid.tile([128, BT], f32r, tag="qT")   # roped+normed q^T
            kT = resid.tile([128, BT], f32r, tag="kT")
            # attention out^T, both heads packed [128, BT]; written via
            # SBUF->SBUF DMA (cross-partition moves are DMA-only)
            yHp = resid.tile([128, BT], bf16, tag="yHp")
            # V in token-major + ones cols: per head h: cols [65h:65h+64]=V_h,
            # col 65h+64 = 1.0
            vA = resid.tile([128, BT // 128, 2 * (HD + 1)], f32r, tag="vA")

            xT_r = xT.ap().rearrange("(o p) t -> p o t", p=128)

            # ================= QKV + RMSNorm + RoPE =================
            xts = {}

            def emit_xt(n):
                tok = slice(512 * n, 512 * n + 512)
                xtA = xtp.tile([128, 4, 512], bf16, tag="xt", name=f"xtA{n}")
                nc.sync.dma_start(xtA[:], xT_r[:, 0:4, tok])
                xtB = xtp.tile([128, 4, 512], bf16, tag="xt", name=f"xtB{n}")
                nc.sync.dma_start(xtB[:], xT_r[:, 4:8, tok])
                xts[n] = (xtA, xtB)

            def emit_qkv(n):
                tok = slice(512 * n, 512 * n + 512)
                ct = slice(512 * (n % 4), 512 * (n % 4) + 512)
                if n not in xts:
                    emit_xt(n)
                xtA, xtB = xts.pop(n)

                bigQK = mmp.tile([128, 1024], f32, tag="big", name=f"qk{n}")
                bigV = mmp.tile([128, 1024], f32, tag="big", name=f"v{n}")
                for m, dst, wcol in ((0, qT, qw_sb), (1, kT, kw_sb), (2, None, None)):
                    ps = bigV[:, 0:512] if m == 2 else bigQK[:, 512 * m:512 * m + 512]
                    for kt in range(C // 128):
                        nc.tensor.matmul(
                            ps,
                            wa_sb[:, kt, 128 * m:128 * m + 128],
                            xtA[:, kt, :] if kt < 4 else xtB[:, kt - 4, :],
                            start=(kt == 0), stop=(kt == C // 128 - 1),
                        )
                    if m == 2:
                        # V: token-major via PE transpose of 128x128 blocks
                        vs = work.tile([128, 512], f32, tag="vs", name=f"vs{n}")
                        nc.scalar.copy(vs[:], ps)
                        for j in range(4):
                            pt = spp.tile([128, 128], f32, tag="sm", name=f"vt{n}_{j}")
                            nc.tensor.transpose(pt[:], vs[:, 128 * j:128 * j + 128],
                                                id_sb[:])
                            kt_g = 4 * n + j
                            nc.vector.tensor_copy(
                                vA[:, kt_g].rearrange("p (h d) -> p h d", h=2)[:, :, 0:HD],
                                pt[:, :].rearrange("p (h d) -> p h d", h=2))
                        continue

                    # stats from raw (pre-weight) psum
                    sq = work.tile([128, 512], f32, tag="scr", name=f"sq{n}_{m}")
                    nc.scalar.activation(r32(sq[:]), ps, AF.Square)
                    ss = spp.tile([2, 512], f32, tag="sm", name=f"ss{n}_{m}")
                    nc.tensor.matmul(ss[:], r32(bo_sb[:]), r32(sq[:]),
                                     start=True, stop=True)
                    inv = work.tile([2, 512], f32, tag="rms", name=f"rms{n}_{m}")
                    nc.scalar.activation(r32(inv[:]), ss[:], AF.Sqrt,
                                         bias=eps_sb[0:2, :], scale=1.0 / HD)
                    with nc.allow_low_precision(reason="f32r is fp32-width"):
                        nc.vector.reciprocal(r32(inv[:]), inv[:])

                    # apply norm weight on the way out of PSUM
                    nc.vector.tensor_scalar_mul(dst[:, tok], ps, wcol[:])

                    # rope: r = q*CS + swap(q)*SN  (swap halves within head)
                    sw = work.tile([128, 512], f32r, tag="sw", name=f"sw{n}_{m}")
                    for h in range(HPC):
                        b0 = 64 * h
                        nc.sync.dma_start(sw[b0:b0 + 32, :], dst[b0 + 32:b0 + 64, tok])
                        nc.sync.dma_start(sw[b0 + 32:b0 + 64, :], dst[b0:b0 + 32, tok])
                    nc.gpsimd.tensor_tensor(sw[:], sw[:], sn_sb[:, ct], MUL)
                    nc.vector.tensor_tensor(dst[:, tok], dst[:, tok], cs_sb[:, ct], MUL)
                    nc.vector.tensor_tensor(dst[:, tok], dst[:, tok], sw[:], ADD)

                    # apply 1/rms: broadcast [2,512] -> [128,512] via K=2 matmul
                    bc = bcp.tile([128, 512], f32, tag="bc", name=f"bc{n}_{m}")
                    nc.tensor.matmul(bc[:], r32(s2_sb[:]), r32(inv[:]),
                                     start=True, stop=True)
                    nc.vector.tensor_tensor(r32(dst[:, tok]), dst[:, tok], bc[:], MUL)

            # ================= causal attention =================
            a_in = dramp.tile([N_CORES, 128, 512], bf16, tag="a_in")

            def emit_attn(b, i):
                if True:
                    qcol = slice(2048 * b + 512 * i, 2048 * b + 512 * i + 512)
                    nkt = 4 * i + 4
                    yps = [ypp.tile([HD + 1, 512], f32, tag="y",
                                    name=f"y{b}_{i}_{h}") for h in range(HPC)]
                    for kt in range(nkt):
                        qs = 128 * (kt - 4 * i) if kt >= 4 * i else 0
                        kc = 2048 * b + 128 * kt
                        kt_g = 16 * b + kt
                        sps = mmp.tile([128, 1024], f32, tag="big",
                                       name=f"s{b}_{i}_{kt}")
                        pT = pwork.tile([128, 1024], f32, tag="pT",
                                        name=f"p{b}_{i}_{kt}")
                        for h in range(HPC):
                            hb = 64 * h
                            nc.tensor.matmul(
                                sps[:, 512 * h + qs:512 * h + 512],
                                r32(kT[hb:hb + 64, kc:kc + 128]),
                                r32(qT[hb:hb + 64, qcol][:, qs:]),
                                start=True, stop=True,
                                tile_position=(hb, 0),
                            )
                        sps3 = sps[:, :].rearrange("p (h q) -> p h q", h=2)[:, :, qs:]
                        pT3 = pT[:, :].rearrange("p (h q) -> p h q", h=2)[:, :, qs:]
                        nc.scalar.activation(r32(pT3), sps3, AF.Exp,
                                             scale=1.0 / 8.0)
                        for h in range(HPC):
                            if kt >= 4 * i:
                                nc.gpsimd.tensor_tensor(
                                    r32(pT[:, 512 * h + qs:512 * h + qs + 128]),
                                    pT[:, 512 * h + qs:512 * h + qs + 128],
                                    wg_sb[:], MUL)
                            nc.tensor.matmul(
                                yps[h][:, qs:],
                                r32(vA[:, kt_g, (HD + 1) * h:(HD + 1) * h + HD + 1]),
                                r32(pT[:, 512 * h + qs:512 * h + 512]),
                                start=(kt == 0), stop=(kt == nkt - 1),
                            )
                    # normalize by the ones-column denominator
                    for h in range(HPC):
                        di = work.tile([1, 512], f32, tag="rms",
                                       name=f"di{b}_{i}_{h}")
                        with nc.allow_low_precision(reason="f32r is fp32-width"):
                            nc.vector.reciprocal(r32(di[:]), yps[h][HD:HD + 1, :])
                        dp = spp.tile([64, 512], f32, tag="sm",
                                      name=f"dp{b}_{i}_{h}")
                        nc.tensor.matmul(dp[:], r32(s2_sb[0:1, 0:64]), r32(di[:]),
                                         start=True, stop=True)
                        dpS = work.tile([64, 512], f32, tag="dpS",
                                        name=f"dpS{b}_{i}_{h}")
                        nc.scalar.copy(dpS[:], dp[:])
                        ybf = work.tile([HD, 512], bf16, tag="ybf",
                                        name=f"ybf{b}_{i}_{h}")
                        nc.vector.tensor_tensor(ybf[:, :],
                                                yps[h][:HD, :], dpS[:, :],
                                                MUL)
                        nc.sync.dma_start(yHp[64 * h:64 * h + HD, qcol],
                                          ybf[:, :])
                    nc.sync.dma_start(a_in[4 * b + i], yHp[:, qcol])

            emit_xt(0)
            emit_late_consts()
            emit_qkv(0)
            for n in range(1, TN // 2):
                emit_qkv(n)
                emit_attn(0, n - 1)
            wp_sb = resid.tile([128, N_CORES, 1024], bf16, tag="wp_sb")
            nc.sync.dma_start(
                wp_sb[:], wpT.ap().rearrange("(o p) f -> p o f", p=128))
            emit_qkv(TN // 2)
            emit_attn(0, 3)
            for n in range(TN // 2 + 1, TN):
                emit_qkv(n)
                emit_attn(1, n - TN // 2 - 1)
            emit_attn(1, 3)

            # ================= AllToAll reshard =================
            a_out = dramp.tile([N_CORES, 128, 512], bf16, tag="a_out")
            if no_cc:
                nc.sync.dma_start(a_out[:, :, :], a_in[:, :, :])
            else:
                nc.gpsimd.collective_compute(
                    "AllToAll", mybir.AluOpType.bypass, replica_groups=RG,
                    ins=[a_in[:, :, :].opt()], outs=[a_out[:, :, :].opt()])


            # ================= c_proj on own 512-token slice =================
            ybr = resid.tile([128, N_CORES, 512], bf16, tag="ybr")
            for r in range(N_CORES):
                nc.sync.dma_start(ybr[:, r, :], a_out[r])
            for cc in range(2):
                ccol = slice(512 * cc, 512 * cc + 512)
                bigP = [mmp.tile([128, 1024], f32, tag="big",
                                 name=f"po{cc}_{t}") for t in range(2)]
                pouts = [bigP[t // 2][:, 512 * (t % 2):512 * (t % 2) + 512]
                         for t in range(4)]
                for r in range(N_CORES):
                    for t in range(4):
                        nc.tensor.matmul(
                            pouts[t],
                            ybr[:, r, 128 * t:128 * t + 128],
                            wp_sb[:, r, ccol],
                            start=(r == 0), stop=(r == N_CORES - 1),
                        )
                for t in range(4):
                    ob = work.tile([128, 512], f32, tag="scr", name=f"ob{cc}_{t}")
                    nc.scalar.copy(ob[:], pouts[t])
                    nc.sync.dma_start(out[128 * t:128 * t + 128, ccol], ob[:])

    nc.compile()
    return nc


def make_in_maps(x, freqs_cos, freqs_sin, w_attn, w_proj, q_norm_w, k_norm_w):
    x = np.asarray(x, np.float32)
    freqs_cos = np.asarray(freqs_cos, np.float32)
    freqs_sin = np.asarray(freqs_sin, np.float32)
    w_attn = np.asarray(w_attn, np.float32)
    w_proj = np.asarray(w_proj, np.float32)
    q_norm_w = np.asarray(q_norm_w, np.float32)
    k_norm_w = np.asarray(k_norm_w, np.float32)

    perm = np.concatenate([np.arange(0, HD, 2), np.arange(1, HD, 2)])
    import ml_dtypes
    xTf = np.ascontiguousarray(x.reshape(BT, C).T.astype(ml_dtypes.bfloat16))
    import ml_dtypes as _md
    wpT = np.ascontiguousarray(w_proj.T.astype(_md.bfloat16))

    cs = np.ascontiguousarray(freqs_cos.T)  # [32, T]
    sn = np.ascontiguousarray(freqs_sin.T)
    csT = np.concatenate([cs, cs] * HPC, axis=0)          # [128, T]
    snT = np.concatenate([-sn, sn] * HPC, axis=0)

    qwc = np.tile(q_norm_w[perm], HPC)[:, None].astype(np.float32)
    kwc = np.tile(k_norm_w[perm], HPC)[:, None].astype(np.float32)

    bones = np.zeros((128, 2), np.float32)
    bones[:64, 0] = 1.0
    bones[64:, 1] = 1.0
    sel2 = np.zeros((2, 128), np.float32)
    sel2[0, :64] = 1.0
    sel2[1, 64:] = 1.0
    wedge = (np.arange(128)[:, None] <= np.arange(128)[None, :]).astype(np.float32)
    vones = np.ones((128, 32), np.float32)
    ident = np.eye(128, dtype=np.float32)

    in_maps = []
    for c in range(N_CORES):
        rows = []
        for sec in range(3):  # q, k, v sections of w_attn
            for h in (HPC * c, HPC * c + 1):
                base = C * sec + HD * h
                if sec < 2:
                    rows.append(base + perm)
                else:
                    rows.append(base + np.arange(HD))
        sel_rows = np.concatenate(rows)
        waT = np.ascontiguousarray(w_attn[sel_rows].T.astype(ml_dtypes.bfloat16))
        in_maps.append({
            "xT": xTf, "waT": waT, "wpT": wpT, "csT": csT, "snT": snT,
            "qw": qwc, "kw": kwc, "bones": bones, "sel2": sel2,
            "wedge": wedge, "ident": ident, "vones": vones,
        })
    return in_maps


_NC_CACHE = {}


def get_nc():
    if "nc" not in _NC_CACHE:
        _NC_CACHE["nc"] = build_nc()
    return _NC_CACHE["nc"]


def kernel(x, freqs_cos, freqs_sin, w_attn, w_proj, q_norm_w, k_norm_w):
    nc = get_nc()
    in_maps = make_in_maps(x, freqs_cos, freqs_sin, w_attn, w_proj,
                           q_norm_w, k_norm_w)
    res = run_bass_kernel_spmd(nc, in_maps, core_ids=list(range(N_CORES)))
    out = np.concatenate([res.results[c]["out"] for c in range(N_CORES)], axis=0)
    return out.reshape(B, T, C).astype(np.float32)

